# revision 1
# baseline (speedup 1.0000x reference)
"""Trainium2 Bass kernel for the chunked GRU-variant scan.

Shapes (hardcoded): x [64, 512, 1024], h0 [64, 1024], W_* [2048, 1024],
b_* [1024]. Output: (outs [64, 128, 4, 1024], finals [64, 128, 1024]).

Strategy: data-parallel over batch across 8 cores (8 rows each, replicated
weights). Phase 1 precomputes A = x @ Wx + b for all three gates as large
float32r GEMMs (full PE rate, FP22 multiply precision). Phase 2 runs the
sequential 512-step scan; per step the recurrent matmuls use the transposed
hidden state as the (tiny, self-loading) stationary operand and stream the
U matrices as the moving operand in N=512 chunks at 1 cycle/row.
"""

import os
import sys

import numpy as np

for _p in ("/opt/trn_rl_repo", "/root/.axon_site/_ro/trn_rl_repo"):
    if os.path.isdir(_p) and _p not in sys.path:
        sys.path.insert(0, _p)

from concourse import bacc, bass_utils, mybir, tile  # noqa: E402

B, T, D, H = 64, 512, 1024, 1024
NCORES = 8
BL = B // NCORES  # batch rows per core
KT = D // 128  # K tiles (8)

_CACHE = {}


def _build(t_steps):
    f32 = mybir.dt.float32
    f32r = mybir.dt.float32r
    AF = mybir.ActivationFunctionType
    ALU = mybir.AluOpType

    nc = bacc.Bacc("TRN2", target_bir_lowering=False, debug=False)

    # Per-core inputs. xT: k-tiles of x-slice transposed, free dim bt=b*T+t.
    xT_d = nc.dram_tensor("xT", [KT, 128, BL * t_steps], f32r, kind="ExternalInput")
    wx_d = nc.dram_tensor("wx", [3, KT, 128, H], f32r, kind="ExternalInput")
    u_d = nc.dram_tensor("u", [KT, 128, 3 * H], f32r, kind="ExternalInput")
    bias_d = nc.dram_tensor("bias", [1, 3 * H], f32r, kind="ExternalInput")
    ones_d = nc.dram_tensor("ones", [1, 128], f32r, kind="ExternalInput")
    ident_d = nc.dram_tensor("ident", [BL, BL], f32, kind="ExternalInput")
    h0T_d = nc.dram_tensor("h0T", [128, KT * BL], f32r, kind="ExternalInput")
    h0_d = nc.dram_tensor("h0bm", [BL, H], f32, kind="ExternalInput")

    A_d = nc.dram_tensor("A", [BL, t_steps, 3 * H], f32)  # internal scratch
    ys_d = nc.dram_tensor("ys", [BL, t_steps, H], f32, kind="ExternalOutput")

    n_tblk = t_steps // 128  # 128-row subtiles per batch row

    with tile.TileContext(nc) as tc:
        # ---------------- Phase 1: A = x @ Wx + b (batch-major) -----------
        with tc.tile_pool(name="p1c", bufs=1) as cpool, \
             tc.tile_pool(name="p1w", bufs=1) as wpool, \
             tc.tile_pool(name="p1x", bufs=2) as xpool, \
             tc.tile_pool(name="p1s", bufs=4) as spool, \
             tc.tile_pool(name="p1ps", bufs=4, space="PSUM") as pspool:
            ones_sb = cpool.tile([1, 128], f32r, tag="ones")
            nc.sync.dma_start(out=ones_sb, in_=ones_d.ap())
            bias_sb = cpool.tile([1, 3 * H], f32r, tag="bias")
            nc.sync.dma_start(out=bias_sb, in_=bias_d.ap())

            for g in range(3):
                wx_sb = wpool.tile([128, KT * H], f32r, tag="wx")
                for k in range(KT):
                    nc.sync.dma_start(
                        out=wx_sb[:, k * H:(k + 1) * H], in_=wx_d.ap()[g, k]
                    )
                for b in range(BL):
                    xt_sb = xpool.tile([128, KT * t_steps], f32r, tag="xt")
                    for k in range(KT):
                        nc.sync.dma_start(
                            out=xt_sb[:, k * t_steps:(k + 1) * t_steps],
                            in_=xT_d.ap()[k, :, b * t_steps:(b + 1) * t_steps],
                        )
                    for sub in range(n_tblk):
                        for n in range(H // 512):
                            ps = pspool.tile([128, 512], f32, tag="ps")
                            for k in range(KT):
                                nc.tensor.matmul(
                                    ps,
                                    lhsT=xt_sb[:, k * t_steps + sub * 128:
                                               k * t_steps + sub * 128 + 128],
                                    rhs=wx_sb[:, k * H + n * 512:
                                              k * H + n * 512 + 512],
                                    start=(k == 0),
                                    stop=False,
                                )
                            nc.tensor.matmul(
                                ps,
                                lhsT=ones_sb,
                                rhs=bias_sb[:, g * H + n * 512:g * H + n * 512 + 512],
                                start=False,
                                stop=True,
                            )
                            cp = spool.tile([128, 512], f32, tag="cp")
                            nc.scalar.activation(out=cp, in_=ps, func=AF.Copy)
                            nc.sync.dma_start(
                                out=A_d.ap()[b, sub * 128:sub * 128 + 128,
                                             g * H + n * 512:g * H + n * 512 + 512],
                                in_=cp,
                            )

        # ---------------- Phase 2: sequential scan ------------------------
        with tc.tile_pool(name="p2c", bufs=1) as c2pool, \
             tc.tile_pool(name="p2u", bufs=1) as upool, \
             tc.tile_pool(name="p2a", bufs=3) as apool, \
             tc.tile_pool(name="p2wk", bufs=2) as wk, \
             tc.tile_pool(name="p2h", bufs=2) as hpool, \
             tc.tile_pool(name="ps_zr", bufs=1, space="PSUM") as pszr, \
             tc.tile_pool(name="ps_hc", bufs=1, space="PSUM") as pshc, \
             tc.tile_pool(name="ps_t1", bufs=1, space="PSUM") as pst1, \
             tc.tile_pool(name="ps_t2", bufs=1, space="PSUM") as pst2:

            u_sb = upool.tile([128, KT * 3 * H], f32r, tag="u")
            for k in range(KT):
                nc.sync.dma_start(
                    out=u_sb[:, k * 3 * H:(k + 1) * 3 * H], in_=u_d.ap()[k]
                )
            ident_sb = c2pool.tile([BL, BL], f32, tag="ident")
            nc.sync.dma_start(out=ident_sb, in_=ident_d.ap())

            hT = hpool.tile([128, KT * BL], f32r, tag="hT")
            nc.sync.dma_start(out=hT, in_=h0T_d.ap())
            h = wk.tile([BL, H], f32, tag="h")
            nc.sync.dma_start(out=h, in_=h0_d.ap())

            def urhs(k, col, width=512):
                base = k * 3 * H + col
                return u_sb[:, base:base + width]

            for t in range(t_steps):
                a_zr = apool.tile([BL, 2 * H], f32, tag="azr")
                nc.sync.dma_start(out=a_zr, in_=A_d.ap()[:, t, 0:2 * H])
                a_h = apool.tile([BL, H], f32, tag="ah")
                nc.sync.dma_start(out=a_h, in_=A_d.ap()[:, t, 2 * H:3 * H])

                psum_zr = pszr.tile([BL, 2 * H], f32, tag="zr")
                # r gate first (columns H..2H of U) so its sigmoid/mult can
                # overlap the z-gate matmuls on the PE.
                for n in (2, 3):
                    for k in range(KT):
                        nc.tensor.matmul(
                            psum_zr[:, n * 512:(n + 1) * 512],
                            lhsT=hT[:, k * BL:(k + 1) * BL],
                            rhs=urhs(k, n * 512),
                            start=(k == 0),
                            stop=(k == KT - 1),
                        )
                nc.vector.tensor_tensor(
                    out=psum_zr[:, H:2 * H], in0=psum_zr[:, H:2 * H],
                    in1=a_zr[:, H:2 * H], op=ALU.add,
                )
                act = wk.tile([BL, 2 * H], f32, tag="act")
                nc.scalar.activation(
                    out=act[:, H:2 * H], in_=psum_zr[:, H:2 * H], func=AF.Sigmoid
                )
                rh = wk.tile([BL, H], f32, tag="rh")
                nc.vector.tensor_tensor(
                    out=rh, in0=act[:, H:2 * H], in1=h, op=ALU.mult
                )

                for n in (0, 1):
                    for k in range(KT):
                        nc.tensor.matmul(
                            psum_zr[:, n * 512:(n + 1) * 512],
                            lhsT=hT[:, k * BL:(k + 1) * BL],
                            rhs=urhs(k, n * 512),
                            start=(k == 0),
                            stop=(k == KT - 1),
                        )
                nc.vector.tensor_tensor(
                    out=psum_zr[:, 0:H], in0=psum_zr[:, 0:H],
                    in1=a_zr[:, 0:H], op=ALU.add,
                )
                nc.scalar.activation(
                    out=act[:, 0:H], in_=psum_zr[:, 0:H], func=AF.Sigmoid
                )

                # transpose r*h into stationary layout
                psT1 = pst1.tile([128, KT * BL], f32, tag="t1")
                for c in range(KT):
                    nc.tensor.transpose(
                        psT1[:, c * BL:(c + 1) * BL],
                        rh[:, c * 128:(c + 1) * 128],
                        ident_sb,
                    )
                rhT = hpool.tile([128, KT * BL], f32r, tag="rhT")
                nc.scalar.activation(out=rhT, in_=psT1, func=AF.Copy)

                psum_hc = pshc.tile([BL, H], f32, tag="hc")
                for n in (0, 1):
                    for k in range(KT):
                        nc.tensor.matmul(
                            psum_hc[:, n * 512:(n + 1) * 512],
                            lhsT=rhT[:, k * BL:(k + 1) * BL],
                            rhs=urhs(k, 2 * H + n * 512),
                            start=(k == 0),
                            stop=(k == KT - 1),
                        )
                nc.vector.tensor_tensor(
                    out=psum_hc, in0=psum_hc, in1=a_h, op=ALU.add
                )
                hc = wk.tile([BL, H], f32, tag="hc")
                nc.scalar.activation(out=hc, in_=psum_hc, func=AF.Tanh)

                # h_new = h + z*(hc - h), reusing hc in place
                nc.vector.tensor_tensor(out=hc, in0=hc, in1=h, op=ALU.subtract)
                nc.vector.tensor_tensor(
                    out=hc, in0=act[:, 0:H], in1=hc, op=ALU.mult
                )
                h_new = wk.tile([BL, H], f32, tag="h")
                nc.vector.tensor_tensor(out=h_new, in0=h, in1=hc, op=ALU.add)

                psT2 = pst2.tile([128, KT * BL], f32, tag="t2")
                for c in range(KT):
                    nc.tensor.transpose(
                        psT2[:, c * BL:(c + 1) * BL],
                        h_new[:, c * 128:(c + 1) * 128],
                        ident_sb,
                    )
                hT_new = hpool.tile([128, KT * BL], f32r, tag="hT")
                nc.scalar.activation(out=hT_new, in_=psT2, func=AF.Copy)

                nc.sync.dma_start(out=ys_d.ap()[:, t, :], in_=h_new)

                h, hT = h_new, hT_new

    nc.compile()
    return nc


def _prep_inputs(x, h0, W_z, b_z, W_r, b_r, W_h, b_h, t_steps):
    f = np.float32
    Wx = np.ascontiguousarray(
        np.stack([W_z[:D], W_r[:D], W_h[:D]]).reshape(3, KT, 128, H), dtype=f
    )
    U = np.concatenate([W_z[D:], W_r[D:], W_h[D:]], axis=1)  # [H, 3H]
    U = np.ascontiguousarray(U.reshape(KT, 128, 3 * H), dtype=f)
    bias = np.concatenate([b_z, b_r, b_h]).astype(f)[None, :]
    ones = np.ones((1, 128), f)
    ident = np.eye(BL, dtype=f)

    in_maps = []
    for c in range(NCORES):
        xc = np.asarray(x[c * BL:(c + 1) * BL, :t_steps], dtype=f)
        xT = np.ascontiguousarray(xc.reshape(BL * t_steps, D).T)  # [D, BL*T]
        xT = xT.reshape(KT, 128, BL * t_steps)
        h0c = np.asarray(h0[c * BL:(c + 1) * BL], dtype=f)
        h0T = np.ascontiguousarray(
            h0c.T.reshape(KT, 128, BL).transpose(1, 0, 2).reshape(128, KT * BL)
        )
        in_maps.append({
            "xT": xT, "wx": Wx, "u": U, "bias": bias, "ones": ones,
            "ident": ident, "h0T": h0T, "h0bm": np.ascontiguousarray(h0c),
        })
    return in_maps


def _run(x, h0, W_z, b_z, W_r, b_r, W_h, b_h, t_steps=T, trace=False):
    if t_steps not in _CACHE:
        _CACHE[t_steps] = _build(t_steps)
    nc = _CACHE[t_steps]
    in_maps = _prep_inputs(x, h0, W_z, b_z, W_r, b_r, W_h, b_h, t_steps)
    res = bass_utils.run_bass_kernel_spmd(
        nc, in_maps, core_ids=list(range(NCORES)), trace=trace
    )
    ys = np.concatenate([res.results[c]["ys"] for c in range(NCORES)], axis=0)
    return ys, res


def kernel(x, h0, W_z, b_z, W_r, b_r, W_h, b_h):
    ys, _ = _run(x, h0, W_z, b_z, W_r, b_r, W_h, b_h)
    outs = ys.reshape(B, T // 4, 4, H)
    finals = np.ascontiguousarray(outs[:, :, -1])
    return outs, finals
